# revision 13
# baseline (speedup 1.0000x reference)
"""Perona-Malik nonlinear diffusion (3 iterations) on Trainium2, 8-core SPMD.

Input : image (16, 1, 1024, 1024) float32
Output: same shape, after 3 iterations of
    g  = Sobel gradients (SAME/zero padding)
    c  = 1 / (1 + |g|^2/kappa^2)   (sqrt folded out algebraically)
    u += DT * div(c * g)           (div via the same Sobel stencils)

Sharding: pure data-parallel, 2 images per NeuronCore.

Per-core kernel: images processed in 9 stripes of 128 rows; all 3
iterations run in SBUF (halo recompute; image top/bottom edges aligned
to tile edges so band-matrix truncation implements the zero padding in
y).  The y-direction Sobel parts are band-matrix matmuls on the tensor
engine accumulating in PSUM; x-direction parts are free-dim shifted
views of zero-padded operand tiles.  All matmul operands are bf16
(full-rate moving operand - fp32r runs at half rate on trn2 - and
fast weight loads); the bands are exact small integers in bf16, and
the running image u is kept in fp32 on the side so bf16 rounding does
not accumulate across iterations (only the conv inputs are quantized).

Engine split per stripe-iteration (all tiles [128, 1024]):
  PE : 20 matmuls (gx 2+2, gy 3+3 halves, divergence 5+5), bf16
  ACT: q1 = Square(gx), q2 = Square(gy), c = Reciprocal(rs*s + rb)
       (square and reciprocal share one activation-table set -> the
        table is loaded once, not swapped per call like ln/exp)
  GPS: s = q1 + q2, and the fp32 -> bf16 re-quantization of u'
  DVE: fx = c*gx, fy = c*gy (scalar_tensor_tensor, bf16 out),
       u' = dv + u (fp32)
All DT/kappa scaling is folded into the Reciprocal's affine pre-scale.

Pipelining: stripes are processed in a rolling 3-deep software pipeline
(emit iter2 of stripe s-2, iter1 of s-1, iter0 of s per step) with PSUM
tags assigned by stripe parity: {gx,dv} and {gy} pairs for even/odd
stripes = exactly 8 banks, so the tensor engine always has an
independent stripe's matmuls to run while another stripe's elementwise
chain completes (keeps the PE HAM clock-gate warm).  Input stripe loads
(SWDGE cast f32->bf16 on gpsimd + plain f32 on sync) are prefetched a
few stripes ahead; output stores ride the sync queue and read from pool
tiles so the ring slot is not recycled before the DMA drains.
"""

import numpy as np

NUM_ITERATIONS = 3
KAPPA = 0.05
DT = 0.25
EPS = 1e-8

N_CORES = 8
IMGS_PER_CORE = 2
H = 1024
W = 1024
P = 128          # partitions / stripe rows
HALO = 6         # 2 rows shrink per iteration * 3 iterations
VALID = P - 2 * HALO          # 116 valid output rows per mid stripe
WPAD = W + 2                  # one zero pad column each side

N_ULOAD = 6      # bf16 input-load ring (loads prefetched LOOKAHEAD ahead)
N_UCAST = 6      # bf16 u' re-quantization ring
N_FBUF = 8
LOOKAHEAD = 3    # stripes of input-DMA prefetch

# knobs (fallbacks if an experiment fails on hw)
USE_ACT_RECIP = True    # False -> nc.vector.reciprocal_approx_fast on DVE
S_ADD_ON_GPS = True     # False -> s-add on DVE
CAST_ON_GPS = False     # False -> u' bf16 cast on ACT

_CACHE = {}


def _band_matrices():
    """[128, 4, 128] float32, index [k, band, m] = lhsT[k, m] of band matrix.

    out[m, :] = sum_k lhsT[k, m] * u[k, :]
      band 0 (Sy):  u[m-1] + 2u[m] + u[m+1]
      band 1 (Syn): -Sy
      band 2 (Dy):  u[m+1] - u[m-1]
      band 3 (Dy2): 2*Dy
    """
    sy = np.zeros((P, P), np.float32)
    dy = np.zeros((P, P), np.float32)
    for m in range(P):
        sy[m, m] = 2.0
        if m - 1 >= 0:
            sy[m - 1, m] = 1.0   # k = m-1
            dy[m - 1, m] = -1.0
        if m + 1 < P:
            sy[m + 1, m] = 1.0   # k = m+1
            dy[m + 1, m] = 1.0
    bands = np.stack([sy, -sy, dy, 2.0 * dy], axis=1)  # [k, 4, m]
    return np.ascontiguousarray(bands.astype(np.float32))


def _stripes():
    """(img, input_row_lo, out_partition_lo, n_out) per stripe."""
    per_img = [(0, 0, P - HALO)]
    pos = P - HALO
    last_start = H - P
    while pos < last_start + HALO:
        n = min(VALID, last_start + HALO - pos)
        per_img.append((pos - HALO, HALO, n))
        pos += n
    per_img.append((last_start, pos - last_start, H - pos))
    return [(img,) + s for img in range(IMGS_PER_CORE) for s in per_img]


def _build_program():
    import concourse.bacc as bacc
    import concourse.tile as tile
    from concourse import mybir

    f32 = mybir.dt.float32
    bf16 = mybir.dt.bfloat16
    AF = mybir.ActivationFunctionType
    OP = mybir.AluOpType

    # c'' = (DT/64) / (1 + (s/64 + eps)/kappa^2)  with s = gx^2+gy^2 in
    # integer-Sobel units (x8 per conv).  1/c'' = s*rs + rb:
    rs = 1.0 / (DT * KAPPA * KAPPA)
    rb = (64.0 / DT) * (1.0 + EPS / (KAPPA * KAPPA))
    # plan-A constants (reciprocal on DVE): x = q1 + q2 + xb with
    # q = (sa*g)^2, then c = 1/x and fx = (c*dt64)*gx.
    sa = 1.0 / (8.0 * KAPPA)
    xb = 1.0 + EPS / (KAPPA * KAPPA)
    dt64 = DT / 64.0

    nc = bacc.Bacc("TRN2", target_bir_lowering=False, debug=False)

    img_d = nc.dram_tensor("image", [IMGS_PER_CORE, H, W], f32, kind="ExternalInput")
    bands_d = nc.dram_tensor("bands", [P, 4, P], f32, kind="ExternalInput")
    zeros_d = nc.dram_tensor("zeros", [P, 1], f32, kind="ExternalInput")
    out_d = nc.dram_tensor("out", [IMGS_PER_CORE, H, W], f32, kind="ExternalOutput")

    # static padded bf16 conv operands (pads zeroed once, never rewritten).
    # Loads and u'-casts use separate rings: a load for stripe s+LOOKAHEAD
    # is emitted before stripe s's compute, so a shared ring would recycle
    # a slot whose reader is not yet emitted (the dependency tracker can
    # only order against already-emitted instructions).
    ul_bufs = [nc.alloc_sbuf_tensor(f"ul{i}", [P, WPAD], bf16).ap()
               for i in range(N_ULOAD)]
    ucast_bufs = [nc.alloc_sbuf_tensor(f"uc{i}", [P, WPAD], bf16).ap()
                  for i in range(N_UCAST)]
    fx_bufs = [nc.alloc_sbuf_tensor(f"fxb{i}", [P, WPAD], bf16).ap()
               for i in range(N_FBUF)]
    fy_bufs = [nc.alloc_sbuf_tensor(f"fyb{i}", [P, WPAD], bf16).ap()
               for i in range(N_FBUF)]

    stripes = _stripes()
    n_stripes = len(stripes)

    with tile.TileContext(nc) as tc:
        with (
            tc.tile_pool(name="const", bufs=1) as const_pool,
            tc.tile_pool(name="ew", bufs=3) as ew_pool,
            tc.tile_pool(name="ps", bufs=1, space="PSUM") as ps_pool,
        ):
            band_t = const_pool.tile([P, 4, P], bf16)
            nc.gpsimd.dma_start(band_t[:], bands_d.ap())

            # zero the pad columns of every padded operand buffer
            # (vector-engine memsets: a pile of tiny SWDGE DMAs would
            # serialize ~40us ahead of the first input loads)
            for buf in ul_bufs + ucast_bufs + fx_bufs + fy_bufs:
                nc.vector.memset(buf[:, 0:1], 0.0)
                nc.vector.memset(buf[:, WPAD - 1:WPAD], 0.0)

            SY = band_t[:, 0, :]
            SYN = band_t[:, 1, :]
            DY = band_t[:, 2, :]
            DY2 = band_t[:, 3, :]

            def conv_mms(psum, terms):
                """psum[:, :] = sum_i band_i @ src_i(x + dx_i), in 512-halves."""
                for h in (0, 512):
                    n = len(terms)
                    for i, (b, src, dx) in enumerate(terms):
                        nc.tensor.matmul(
                            psum[:, h:h + 512], b,
                            src[:, h + dx + 1:h + dx + 513],
                            start=(i == 0), stop=(i == n - 1))

            lc = 0   # load ring counter
            cc = 0   # cast ring counter
            fc = 0   # flux buffer rotation counter
            state = {}   # stripe index -> (bf16 conv input, fp32 u tile)
            pend = {}    # (stripe, iter) -> tiles dict between phases

            def load_stripe(s):
                nonlocal lc
                img, in_lo, _, _ = stripes[s]
                u_b = ul_bufs[lc % N_ULOAD]; lc += 1
                nc.gpsimd.dma_start(u_b[:, 1:W + 1],
                                    img_d.ap()[img, in_lo:in_lo + P, :])
                u_f = ew_pool.tile([P, W], f32, tag="uin", bufs=6,
                                   name="uin")
                nc.sync.dma_start(u_f[:], img_d.ap()[img, in_lo:in_lo + P, :])
                state[s] = (u_b, u_f)

            def do_a1(s, it):
                """gradient matmuls + squares + s-add."""
                par = s % 2
                u_b, u_f = state[s]

                gx = ps_pool.tile([P, W], f32, tag=f"g{par}", name=f"gx{par}")
                gy = ps_pool.tile([P, W], f32, tag=f"y{par}", name=f"gy{par}")
                conv_mms(gx, [(SY, u_b, 1), (SYN, u_b, -1)])
                conv_mms(gy, [(DY2, u_b, 0), (DY, u_b, -1), (DY, u_b, 1)])

                q1 = ew_pool.tile([P, W], f32, tag="q1", bufs=4)
                q2 = ew_pool.tile([P, W], f32, tag="q2", bufs=4)
                s_t = ew_pool.tile([P, W], f32, tag="s", bufs=4)
                # the s-add sits inside the gx/gy PSUM-residency window, so
                # keep it on the faster DVE
                add_eng = nc.vector
                if USE_ACT_RECIP:
                    nc.scalar.activation(q1[:], gx[:], AF.Square)
                    nc.scalar.activation(q2[:], gy[:], AF.Square)
                    add_eng.tensor_tensor(s_t[:], q1[:], q2[:], OP.add)
                else:
                    nc.scalar.activation(q1[:], gx[:], AF.Square, scale=sa)
                    nc.scalar.activation(q2[:], gy[:], AF.Square, scale=sa)
                    add_eng.scalar_tensor_tensor(
                        s_t[:], q1[:], xb, q2[:], op0=OP.add, op1=OP.add)
                pend[(s, it)] = dict(gx=gx, gy=gy, s_t=s_t)

            def do_a2(s, it):
                """reciprocal + fluxes (consumes gx/gy from psum)."""
                nonlocal fc
                p = pend[(s, it)]
                c_t = ew_pool.tile([P, W], f32, tag="c", bufs=4)
                if USE_ACT_RECIP:
                    _act_recip(nc, c_t[:], p["s_t"][:], scale=rs, bias=rb)
                    flux_s = 1.0
                else:
                    nc.vector.reciprocal_approx_fast(c_t[:], p["s_t"][:])
                    flux_s = dt64
                fx = fx_bufs[fc % N_FBUF]
                fy = fy_bufs[fc % N_FBUF]; fc += 1
                nc.vector.scalar_tensor_tensor(
                    fx[:, 1:W + 1], c_t[:], flux_s, p["gx"][:],
                    op0=OP.mult, op1=OP.mult)
                nc.vector.scalar_tensor_tensor(
                    fy[:, 1:W + 1], c_t[:], flux_s, p["gy"][:],
                    op0=OP.mult, op1=OP.mult)
                p["fx"] = fx; p["fy"] = fy

            def do_b_div(s, it):
                """divergence matmuls (consume the fluxes)."""
                par = s % 2
                p = pend[(s, it)]
                # dv reuses gx's psum slot (same tag, freed by the fx read)
                dv = ps_pool.tile([P, W], f32, tag=f"g{par}", name=f"dv{par}")
                conv_mms(dv, [(SY, p["fx"], 1), (SYN, p["fx"], -1),
                              (DY2, p["fy"], 0), (DY, p["fy"], -1),
                              (DY, p["fy"], 1)])
                p["dv"] = dv

            def do_b_fin(s, it):
                """u update + bf16 requant (or store)."""
                nonlocal cc
                img, in_lo, op_lo, n_out = stripes[s]
                _, u_f = state[s]
                p = pend.pop((s, it))
                dv = p["dv"]

                if it < NUM_ITERATIONS - 1:
                    u_n = ew_pool.tile([P, W], f32, tag="unew", bufs=6,
                                       name="unew")
                    nc.vector.scalar_tensor_tensor(
                        u_n[:], dv[:], 1.0, u_f[:], op0=OP.mult, op1=OP.add)
                    u_nb = ucast_bufs[cc % N_UCAST]; cc += 1
                    # fp32->bf16 requant as a cast-DMA: zero engine time,
                    # and the extra transfer latency is off the PSUM
                    # residency span (only on the iter-to-iter path)
                    nc.gpsimd.dma_start(u_nb[:, 1:W + 1], u_n[:])
                    state[s] = (u_nb, u_n)
                else:
                    # final iteration: fp32 result into a pool tile (the
                    # ring tracks the store DMA as a reader, so the slot
                    # is not recycled before the transfer completed)
                    u_o = ew_pool.tile([P, W], f32, tag="uout", bufs=6,
                                       name="uout")
                    nc.vector.scalar_tensor_tensor(
                        u_o[:], dv[:], 1.0, u_f[:], op0=OP.mult, op1=OP.add)
                    r0 = in_lo + op_lo
                    nc.sync.dma_start(
                        out_d.ap()[img, r0:r0 + n_out, :],
                        u_o[op_lo:op_lo + n_out, :])

            # rolling pipeline: per step, divergence phases of last step's
            # chains first (their fluxes are ready), then gradient phases
            # of this step's chains, then reciprocal+flux phases -- keeps
            # every engine queue free of head-of-line blocking.
            def valid(s, it):
                return 0 <= s < n_stripes
            for s in range(min(LOOKAHEAD, n_stripes)):
                load_stripe(s)
            for k in range(n_stripes + 3):
                b_list = [(s, it) for (s, it) in
                          [(k - 3, 2), (k - 2, 1), (k - 1, 0)]
                          if valid(s, it) and (s, it) in pend]
                for (s, it) in b_list:
                    do_b_div(s, it)
                for (s, it) in b_list:
                    do_b_fin(s, it)
                a_list = [(s, it) for (s, it) in
                          [(k - 2, 2), (k - 1, 1), (k, 0)] if valid(s, it)]
                a_list = [(s, it) for (s, it) in a_list
                          if it == 0 or (s, it - 1) not in pend]
                for (s, it) in a_list:
                    do_a1(s, it)
                if k + LOOKAHEAD < n_stripes:
                    load_stripe(k + LOOKAHEAD)
                for (s, it) in a_list:
                    do_a2(s, it)

    nc.compile()
    return nc


def _act_recip(nc, out_ap, in_ap, scale, bias):
    """c = 1/(scale*in + bias) on the scalar engine.

    bass's activation() wrapper refuses Reciprocal outright (a generic
    accuracy warning); the LUT's ~1e-5 relative error (measured on hw
    for this input range) is far inside this problem's tolerance, so
    emit the InstActivation directly.
    """
    from concourse import mybir

    eng = nc.scalar
    imm = lambda v: mybir.ImmediateValue(dtype=mybir.dt.float32, value=float(v))
    return eng.add_instruction(
        mybir.InstActivation(
            name=eng.bass.get_next_instruction_name(),
            func=mybir.ActivationFunctionType.Reciprocal,
            ins=[eng.lower_ap(in_ap), imm(bias), imm(scale), imm(0.0)],
            outs=[eng.lower_ap(out_ap)],
        )
    )


def _get_program():
    if "nc" not in _CACHE:
        _CACHE["nc"] = _build_program()
        _CACHE["bands"] = _band_matrices()
        _CACHE["zeros"] = np.zeros((P, 1), np.float32)
    return _CACHE["nc"], _CACHE["bands"], _CACHE["zeros"]


def kernel(image):
    from concourse.bass_utils import run_bass_kernel_spmd

    image = np.asarray(image)
    orig_shape = image.shape          # (16, 1, 1024, 1024)
    flat = np.ascontiguousarray(
        image.reshape(N_CORES, IMGS_PER_CORE, H, W).astype(np.float32))

    nc, bands, zeros = _get_program()
    in_maps = [{"image": flat[c], "bands": bands, "zeros": zeros}
               for c in range(N_CORES)]
    res = run_bass_kernel_spmd(nc, in_maps, core_ids=list(range(N_CORES)))
    out = np.stack([res.results[c]["out"] for c in range(N_CORES)])
    return out.reshape(orig_shape).astype(np.float32)


# revision 14
# speedup vs baseline: 1.2781x; 1.2781x over previous
"""Perona-Malik nonlinear diffusion (3 iterations) on Trainium2, 8-core SPMD.

Input : image (16, 1, 1024, 1024) float32
Output: same shape, after 3 iterations of
    g  = Sobel gradients (SAME/zero padding)
    c  = 1 / (1 + |g|^2/kappa^2)   (sqrt folded out algebraically)
    u += DT * div(c * g)           (div via the same Sobel stencils)

Sharding: pure data-parallel, 2 images per NeuronCore.

Per-core kernel: images processed in 9 stripes of 128 rows; all 3
iterations run in SBUF (halo recompute; image top/bottom edges aligned
to tile edges so band-matrix truncation implements the zero padding in
y).  The y-direction Sobel parts are band-matrix matmuls on the tensor
engine accumulating in PSUM; x-direction parts are free-dim shifted
views of zero-padded operand tiles.  All matmul operands are bf16
(full-rate moving operand - fp32r runs at half rate on trn2 - and
fast weight loads); the bands are exact small integers in bf16, and
the running image u is kept in fp32 on the side so bf16 rounding does
not accumulate across iterations (only the conv inputs are quantized).

Engine split per stripe-iteration (all tiles [128, 1024]):
  PE : 20 matmuls (gx 2+2, gy 3+3 halves, divergence 5+5), bf16
  ACT: q1 = Square(gx), q2 = Square(gy), c = Reciprocal(rs*s + rb)
       (square and reciprocal share one activation-table set -> the
        table is loaded once, not swapped per call like ln/exp)
  GPS: s = q1 + q2, and the fp32 -> bf16 re-quantization of u'
  DVE: fx = c*gx, fy = c*gy (scalar_tensor_tensor, bf16 out),
       u' = dv + u (fp32)
All DT/kappa scaling is folded into the Reciprocal's affine pre-scale.

Pipelining: stripes are processed in a rolling 3-deep software pipeline
(emit iter2 of stripe s-2, iter1 of s-1, iter0 of s per step) with PSUM
tags assigned by stripe parity: {gx,dv} and {gy} pairs for even/odd
stripes = exactly 8 banks, so the tensor engine always has an
independent stripe's matmuls to run while another stripe's elementwise
chain completes (keeps the PE HAM clock-gate warm).  Input stripe loads
(SWDGE cast f32->bf16 on gpsimd + plain f32 on sync) are prefetched a
few stripes ahead; output stores ride the sync queue and read from pool
tiles so the ring slot is not recycled before the DMA drains.
"""

import numpy as np

NUM_ITERATIONS = 3
KAPPA = 0.05
DT = 0.25
EPS = 1e-8

N_CORES = 8
IMGS_PER_CORE = 2
H = 1024
W = 1024
P = 128          # partitions / stripe rows
HALO = 6         # 2 rows shrink per iteration * 3 iterations
VALID = P - 2 * HALO          # 116 valid output rows per mid stripe
WPAD = W + 2                  # one zero pad column each side

N_ULOAD = 6      # bf16 input-load ring (loads prefetched LOOKAHEAD ahead)
N_UCAST = 6      # bf16 u' re-quantization ring
N_FBUF = 8
LOOKAHEAD = 3    # stripes of input-DMA prefetch

# knobs (fallbacks if an experiment fails on hw)
USE_ACT_RECIP = True    # False -> nc.vector.reciprocal_approx_fast on DVE
S_ADD_ON_GPS = True     # False -> s-add on DVE
CAST_ON_GPS = False     # False -> u' bf16 cast on ACT

_CACHE = {}


def _band_matrices():
    """[128, 4, 128] float32, index [k, band, m] = lhsT[k, m] of band matrix.

    out[m, :] = sum_k lhsT[k, m] * u[k, :]
      band 0 (Sy):  u[m-1] + 2u[m] + u[m+1]
      band 1 (Syn): -Sy
      band 2 (Dy):  u[m+1] - u[m-1]
      band 3 (Dy2): 2*Dy
    """
    sy = np.zeros((P, P), np.float32)
    dy = np.zeros((P, P), np.float32)
    for m in range(P):
        sy[m, m] = 2.0
        if m - 1 >= 0:
            sy[m - 1, m] = 1.0   # k = m-1
            dy[m - 1, m] = -1.0
        if m + 1 < P:
            sy[m + 1, m] = 1.0   # k = m+1
            dy[m + 1, m] = 1.0
    bands = np.stack([sy, -sy, dy, 2.0 * dy], axis=1)  # [k, 4, m]
    return np.ascontiguousarray(bands.astype(np.float32))


def _stripes():
    """(img, input_row_lo, out_partition_lo, n_out) per stripe."""
    per_img = [(0, 0, P - HALO)]
    pos = P - HALO
    last_start = H - P
    while pos < last_start + HALO:
        n = min(VALID, last_start + HALO - pos)
        per_img.append((pos - HALO, HALO, n))
        pos += n
    per_img.append((last_start, pos - last_start, H - pos))
    return [(img,) + s for img in range(IMGS_PER_CORE) for s in per_img]


def _build_program():
    import concourse.bacc as bacc
    import concourse.tile as tile
    from concourse import mybir

    f32 = mybir.dt.float32
    bf16 = mybir.dt.bfloat16
    AF = mybir.ActivationFunctionType
    OP = mybir.AluOpType

    # c'' = (DT/64) / (1 + (s/64 + eps)/kappa^2)  with s = gx^2+gy^2 in
    # integer-Sobel units (x8 per conv).  1/c'' = s*rs + rb:
    rs = 1.0 / (DT * KAPPA * KAPPA)
    rb = (64.0 / DT) * (1.0 + EPS / (KAPPA * KAPPA))
    # plan-A constants (reciprocal on DVE): x = q1 + q2 + xb with
    # q = (sa*g)^2, then c = 1/x and fx = (c*dt64)*gx.
    sa = 1.0 / (8.0 * KAPPA)
    xb = 1.0 + EPS / (KAPPA * KAPPA)
    dt64 = DT / 64.0

    nc = bacc.Bacc("TRN2", target_bir_lowering=False, debug=False)

    img_d = nc.dram_tensor("image", [IMGS_PER_CORE, H, W], f32, kind="ExternalInput")
    bands_d = nc.dram_tensor("bands", [P, 4, P], f32, kind="ExternalInput")
    zeros_d = nc.dram_tensor("zeros", [P, 1], f32, kind="ExternalInput")
    out_d = nc.dram_tensor("out", [IMGS_PER_CORE, H, W], f32, kind="ExternalOutput")

    # static padded bf16 conv operands (pads zeroed once, never rewritten).
    # Loads and u'-casts use separate rings: a load for stripe s+LOOKAHEAD
    # is emitted before stripe s's compute, so a shared ring would recycle
    # a slot whose reader is not yet emitted (the dependency tracker can
    # only order against already-emitted instructions).
    ul_bufs = [nc.alloc_sbuf_tensor(f"ul{i}", [P, WPAD], bf16).ap()
               for i in range(N_ULOAD)]
    ucast_bufs = [nc.alloc_sbuf_tensor(f"uc{i}", [P, WPAD], bf16).ap()
                  for i in range(N_UCAST)]
    fx_bufs = [nc.alloc_sbuf_tensor(f"fxb{i}", [P, WPAD], bf16).ap()
               for i in range(N_FBUF)]
    fy_bufs = [nc.alloc_sbuf_tensor(f"fyb{i}", [P, WPAD], bf16).ap()
               for i in range(N_FBUF)]

    stripes = _stripes()
    n_stripes = len(stripes)

    with tile.TileContext(nc) as tc:
        with (
            tc.tile_pool(name="const", bufs=1) as const_pool,
            tc.tile_pool(name="ew", bufs=3) as ew_pool,
            tc.tile_pool(name="ps", bufs=1, space="PSUM") as ps_pool,
        ):
            band_t = const_pool.tile([P, 4, P], bf16)
            nc.gpsimd.dma_start(band_t[:], bands_d.ap())

            # zero the pad columns of every padded operand buffer
            # (vector-engine memsets: a pile of tiny SWDGE DMAs would
            # serialize ~40us ahead of the first input loads)
            for buf in ul_bufs + ucast_bufs + fx_bufs + fy_bufs:
                nc.vector.memset(buf[:, 0:1], 0.0)
                nc.vector.memset(buf[:, WPAD - 1:WPAD], 0.0)

            SY = band_t[:, 0, :]
            SYN = band_t[:, 1, :]
            DY = band_t[:, 2, :]
            DY2 = band_t[:, 3, :]

            def conv_mms(psum, terms):
                """psum[:, :] = sum_i band_i @ src_i(x + dx_i), in 512-halves."""
                for h in (0, 512):
                    n = len(terms)
                    for i, (b, src, dx) in enumerate(terms):
                        nc.tensor.matmul(
                            psum[:, h:h + 512], b,
                            src[:, h + dx + 1:h + dx + 513],
                            start=(i == 0), stop=(i == n - 1))

            lc = 0   # load ring counter
            cc = 0   # cast ring counter
            fc = 0   # flux buffer rotation counter
            state = {}   # stripe index -> (bf16 conv input, fp32 u tile)
            pend = {}    # (stripe, iter) -> tiles dict between phases

            def load_stripe(s):
                nonlocal lc
                img, in_lo, _, _ = stripes[s]
                u_b = ul_bufs[lc % N_ULOAD]; lc += 1
                nc.gpsimd.dma_start(u_b[:, 1:W + 1],
                                    img_d.ap()[img, in_lo:in_lo + P, :])
                u_f = ew_pool.tile([P, W], f32, tag="uin", bufs=6,
                                   name="uin")
                nc.sync.dma_start(u_f[:], img_d.ap()[img, in_lo:in_lo + P, :])
                state[s] = (u_b, u_f)

            def do_a1(s, it):
                """gradient matmuls + squares + s-add."""
                par = s % 2
                u_b, u_f = state[s]

                gx = ps_pool.tile([P, W], f32, tag=f"g{par}", name=f"gx{par}")
                gy = ps_pool.tile([P, W], f32, tag=f"y{par}", name=f"gy{par}")
                conv_mms(gx, [(SY, u_b, 1), (SYN, u_b, -1)])
                conv_mms(gy, [(DY2, u_b, 0), (DY, u_b, -1), (DY, u_b, 1)])

                q1 = ew_pool.tile([P, W], f32, tag="q1", bufs=4)
                q2 = ew_pool.tile([P, W], f32, tag="q2", bufs=4)
                s_t = ew_pool.tile([P, W], f32, tag="s", bufs=4)
                # the s-add sits inside the gx/gy PSUM-residency window, so
                # keep it on the faster DVE
                add_eng = nc.vector
                if USE_ACT_RECIP:
                    nc.scalar.activation(q1[:], gx[:], AF.Square)
                    nc.scalar.activation(q2[:], gy[:], AF.Square)
                    add_eng.tensor_tensor(s_t[:], q1[:], q2[:], OP.add)
                else:
                    nc.scalar.activation(q1[:], gx[:], AF.Square, scale=sa)
                    nc.scalar.activation(q2[:], gy[:], AF.Square, scale=sa)
                    add_eng.scalar_tensor_tensor(
                        s_t[:], q1[:], xb, q2[:], op0=OP.add, op1=OP.add)
                pend[(s, it)] = dict(gx=gx, gy=gy, s_t=s_t)

            def do_a2(s, it):
                """reciprocal + fluxes (consumes gx/gy from psum)."""
                nonlocal fc
                p = pend[(s, it)]
                c_t = ew_pool.tile([P, W], f32, tag="c", bufs=4)
                if USE_ACT_RECIP:
                    _act_recip(nc, c_t[:], p["s_t"][:], scale=rs, bias=rb)
                    flux_s = 1.0
                else:
                    nc.vector.reciprocal_approx_fast(c_t[:], p["s_t"][:])
                    flux_s = dt64
                fx = fx_bufs[fc % N_FBUF]
                fy = fy_bufs[fc % N_FBUF]; fc += 1
                nc.vector.scalar_tensor_tensor(
                    fx[:, 1:W + 1], c_t[:], flux_s, p["gx"][:],
                    op0=OP.mult, op1=OP.mult)
                nc.vector.scalar_tensor_tensor(
                    fy[:, 1:W + 1], c_t[:], flux_s, p["gy"][:],
                    op0=OP.mult, op1=OP.mult)
                p["fx"] = fx; p["fy"] = fy

            def do_b_div(s, it):
                """divergence matmuls (consume the fluxes)."""
                par = s % 2
                p = pend[(s, it)]
                # dv reuses gx's psum slot (same tag, freed by the fx read)
                dv = ps_pool.tile([P, W], f32, tag=f"g{par}", name=f"dv{par}")
                conv_mms(dv, [(SY, p["fx"], 1), (SYN, p["fx"], -1),
                              (DY2, p["fy"], 0), (DY, p["fy"], -1),
                              (DY, p["fy"], 1)])
                p["dv"] = dv

            def do_b_fin(s, it):
                """u update + bf16 requant (or store)."""
                nonlocal cc
                img, in_lo, op_lo, n_out = stripes[s]
                _, u_f = state[s]
                p = pend.pop((s, it))
                dv = p["dv"]

                if it < NUM_ITERATIONS - 1:
                    u_n = ew_pool.tile([P, W], f32, tag="unew", bufs=6,
                                       name="unew")
                    nc.vector.scalar_tensor_tensor(
                        u_n[:], dv[:], 1.0, u_f[:], op0=OP.mult, op1=OP.add)
                    u_nb = ucast_bufs[cc % N_UCAST]; cc += 1
                    nc.scalar.copy(u_nb[:, 1:W + 1], u_n[:])
                    state[s] = (u_nb, u_n)
                else:
                    # final iteration: fp32 result into a pool tile (the
                    # ring tracks the store DMA as a reader, so the slot
                    # is not recycled before the transfer completed)
                    u_o = ew_pool.tile([P, W], f32, tag="uout", bufs=6,
                                       name="uout")
                    nc.vector.scalar_tensor_tensor(
                        u_o[:], dv[:], 1.0, u_f[:], op0=OP.mult, op1=OP.add)
                    r0 = in_lo + op_lo
                    nc.sync.dma_start(
                        out_d.ap()[img, r0:r0 + n_out, :],
                        u_o[op_lo:op_lo + n_out, :])

            # rolling pipeline: per step, divergence phases of last step's
            # chains first (their fluxes are ready), then gradient phases
            # of this step's chains, then reciprocal+flux phases -- keeps
            # every engine queue free of head-of-line blocking.
            def valid(s, it):
                return 0 <= s < n_stripes
            for s in range(min(LOOKAHEAD, n_stripes)):
                load_stripe(s)
            for k in range(n_stripes + 3):
                b_list = [(s, it) for (s, it) in
                          [(k - 3, 2), (k - 2, 1), (k - 1, 0)]
                          if valid(s, it) and (s, it) in pend]
                for (s, it) in b_list:
                    do_b_div(s, it)
                for (s, it) in b_list:
                    do_b_fin(s, it)
                a_list = [(s, it) for (s, it) in
                          [(k - 2, 2), (k - 1, 1), (k, 0)] if valid(s, it)]
                a_list = [(s, it) for (s, it) in a_list
                          if it == 0 or (s, it - 1) not in pend]
                for (s, it) in a_list:
                    do_a1(s, it)
                if k + LOOKAHEAD < n_stripes:
                    load_stripe(k + LOOKAHEAD)
                for (s, it) in a_list:
                    do_a2(s, it)

    nc.compile()
    return nc


def _act_recip(nc, out_ap, in_ap, scale, bias):
    """c = 1/(scale*in + bias) on the scalar engine.

    bass's activation() wrapper refuses Reciprocal outright (a generic
    accuracy warning); the LUT's ~1e-5 relative error (measured on hw
    for this input range) is far inside this problem's tolerance, so
    emit the InstActivation directly.
    """
    from concourse import mybir

    eng = nc.scalar
    imm = lambda v: mybir.ImmediateValue(dtype=mybir.dt.float32, value=float(v))
    return eng.add_instruction(
        mybir.InstActivation(
            name=eng.bass.get_next_instruction_name(),
            func=mybir.ActivationFunctionType.Reciprocal,
            ins=[eng.lower_ap(in_ap), imm(bias), imm(scale), imm(0.0)],
            outs=[eng.lower_ap(out_ap)],
        )
    )


def _get_program():
    if "nc" not in _CACHE:
        _CACHE["nc"] = _build_program()
        _CACHE["bands"] = _band_matrices()
        _CACHE["zeros"] = np.zeros((P, 1), np.float32)
    return _CACHE["nc"], _CACHE["bands"], _CACHE["zeros"]


def kernel(image):
    from concourse.bass_utils import run_bass_kernel_spmd

    image = np.asarray(image)
    orig_shape = image.shape          # (16, 1, 1024, 1024)
    flat = np.ascontiguousarray(
        image.reshape(N_CORES, IMGS_PER_CORE, H, W).astype(np.float32))

    nc, bands, zeros = _get_program()
    in_maps = [{"image": flat[c], "bands": bands, "zeros": zeros}
               for c in range(N_CORES)]
    res = run_bass_kernel_spmd(nc, in_maps, core_ids=list(range(N_CORES)))
    out = np.stack([res.results[c]["out"] for c in range(N_CORES)])
    return out.reshape(orig_shape).astype(np.float32)


# revision 15
# speedup vs baseline: 1.9064x; 1.4916x over previous
"""Perona-Malik nonlinear diffusion (3 iterations) on Trainium2, 8-core SPMD.

Input : image (16, 1, 1024, 1024) float32
Output: same shape, after 3 iterations of
    g  = Sobel gradients (SAME/zero padding)
    c  = 1 / (1 + |g|^2/kappa^2)   (sqrt folded out algebraically)
    u += DT * div(c * g)           (div via the same Sobel stencils)

Sharding: pure data-parallel, 2 images per NeuronCore.

Per-core kernel: images processed in 9 stripes of 128 rows; all 3
iterations run in SBUF (halo recompute; image top/bottom edges aligned
to tile edges so band-matrix truncation implements the zero padding in
y).  The y-direction Sobel parts are band-matrix matmuls on the tensor
engine accumulating in PSUM; x-direction parts are free-dim shifted
views of zero-padded operand tiles.  All matmul operands are bf16
(full-rate moving operand - fp32r runs at half rate on trn2 - and
fast weight loads); the bands are exact small integers in bf16, and
the running image u is kept in fp32 on the side so bf16 rounding does
not accumulate across iterations (only the conv inputs are quantized).

Engine split per stripe-iteration (all tiles [128, 1024]):
  PE : 20 matmuls (gx 2+2, gy 3+3 halves, divergence 5+5), bf16
  ACT: q1 = Square(gx), q2 = Square(gy), c = Reciprocal(rs*s + rb)
       (square and reciprocal share one activation-table set -> the
        table is loaded once, not swapped per call like ln/exp)
  GPS: s = q1 + q2, and the fp32 -> bf16 re-quantization of u'
  DVE: fx = c*gx, fy = c*gy (scalar_tensor_tensor, bf16 out),
       u' = dv + u (fp32)
All DT/kappa scaling is folded into the Reciprocal's affine pre-scale.

Pipelining: stripes are processed in a rolling 3-deep software pipeline
(emit iter2 of stripe s-2, iter1 of s-1, iter0 of s per step) with PSUM
tags assigned by stripe parity: {gx,dv} and {gy} pairs for even/odd
stripes = exactly 8 banks, so the tensor engine always has an
independent stripe's matmuls to run while another stripe's elementwise
chain completes (keeps the PE HAM clock-gate warm).  Input stripe loads
(SWDGE cast f32->bf16 on gpsimd + plain f32 on sync) are prefetched a
few stripes ahead; output stores ride the sync queue and read from pool
tiles so the ring slot is not recycled before the DMA drains.
"""

import numpy as np

NUM_ITERATIONS = 3
KAPPA = 0.05
DT = 0.25
EPS = 1e-8

N_CORES = 8
IMGS_PER_CORE = 2
H = 1024
W = 1024
P = 128          # partitions / stripe rows
HALO = 6         # 2 rows shrink per iteration * 3 iterations
VALID = P - 2 * HALO          # 116 valid output rows per mid stripe
WPAD = W + 2                  # one zero pad column each side

N_ULOAD = 6      # bf16 input-load ring (loads prefetched LOOKAHEAD ahead)
N_UCAST = 6      # bf16 u' re-quantization ring
N_FBUF = 8
LOOKAHEAD = 3    # stripes of input-DMA prefetch

# knobs (fallbacks if an experiment fails on hw)
USE_ACT_RECIP = True    # False -> nc.vector.reciprocal_approx_fast on DVE
S_ADD_ON_GPS = True     # False -> s-add on DVE
CAST_ON_GPS = False     # False -> u' bf16 cast on ACT

_CACHE = {}


def _band_matrices():
    """[128, 4, 128] float32, index [k, band, m] = lhsT[k, m] of band matrix.

    out[m, :] = sum_k lhsT[k, m] * u[k, :]
      band 0 (Sy):  u[m-1] + 2u[m] + u[m+1]
      band 1 (Syn): -Sy
      band 2 (Dy):  u[m+1] - u[m-1]
      band 3 (Dy2): 2*Dy
    """
    sy = np.zeros((P, P), np.float32)
    dy = np.zeros((P, P), np.float32)
    for m in range(P):
        sy[m, m] = 2.0
        if m - 1 >= 0:
            sy[m - 1, m] = 1.0   # k = m-1
            dy[m - 1, m] = -1.0
        if m + 1 < P:
            sy[m + 1, m] = 1.0   # k = m+1
            dy[m + 1, m] = 1.0
    bands = np.stack([sy, -sy, dy, 2.0 * dy], axis=1)  # [k, 4, m]
    return np.ascontiguousarray(bands.astype(np.float32))


def _stripes():
    """(img, input_row_lo, out_partition_lo, n_out) per stripe."""
    per_img = [(0, 0, P - HALO)]
    pos = P - HALO
    last_start = H - P
    while pos < last_start + HALO:
        n = min(VALID, last_start + HALO - pos)
        per_img.append((pos - HALO, HALO, n))
        pos += n
    per_img.append((last_start, pos - last_start, H - pos))
    return [(img,) + s for img in range(IMGS_PER_CORE) for s in per_img]


def _build_program():
    import concourse.bacc as bacc
    import concourse.tile as tile
    from concourse import mybir

    f32 = mybir.dt.float32
    bf16 = mybir.dt.bfloat16
    AF = mybir.ActivationFunctionType
    OP = mybir.AluOpType

    # c'' = (DT/64) / (1 + (s/64 + eps)/kappa^2)  with s = gx^2+gy^2 in
    # integer-Sobel units (x8 per conv).  1/c'' = s*rs + rb:
    rs = 1.0 / (DT * KAPPA * KAPPA)
    rb = (64.0 / DT) * (1.0 + EPS / (KAPPA * KAPPA))
    # plan-A constants (reciprocal on DVE): x = q1 + q2 + xb with
    # q = (sa*g)^2, then c = 1/x and fx = (c*dt64)*gx.
    sa = 1.0 / (8.0 * KAPPA)
    xb = 1.0 + EPS / (KAPPA * KAPPA)
    dt64 = DT / 64.0

    nc = bacc.Bacc("TRN2", target_bir_lowering=False, debug=False)

    img_d = nc.dram_tensor("image", [IMGS_PER_CORE, H, W], f32, kind="ExternalInput")
    bands_d = nc.dram_tensor("bands", [P, 4, P], f32, kind="ExternalInput")
    zeros_d = nc.dram_tensor("zeros", [P, 1], f32, kind="ExternalInput")
    out_d = nc.dram_tensor("out", [IMGS_PER_CORE, H, W], f32, kind="ExternalOutput")

    # static padded bf16 conv operands (pads zeroed once, never rewritten).
    # Loads and u'-casts use separate rings: a load for stripe s+LOOKAHEAD
    # is emitted before stripe s's compute, so a shared ring would recycle
    # a slot whose reader is not yet emitted (the dependency tracker can
    # only order against already-emitted instructions).
    ul_bufs = [nc.alloc_sbuf_tensor(f"ul{i}", [P, WPAD], bf16).ap()
               for i in range(N_ULOAD)]
    ucast_bufs = [nc.alloc_sbuf_tensor(f"uc{i}", [P, WPAD], bf16).ap()
                  for i in range(N_UCAST)]
    fx_bufs = [nc.alloc_sbuf_tensor(f"fxb{i}", [P, WPAD], bf16).ap()
               for i in range(N_FBUF)]
    fy_bufs = [nc.alloc_sbuf_tensor(f"fyb{i}", [P, WPAD], bf16).ap()
               for i in range(N_FBUF)]

    stripes = _stripes()
    n_stripes = len(stripes)

    with tile.TileContext(nc) as tc:
        with (
            tc.tile_pool(name="const", bufs=1) as const_pool,
            tc.tile_pool(name="ew", bufs=3) as ew_pool,
            tc.tile_pool(name="ps", bufs=1, space="PSUM") as ps_pool,
        ):
            band_t = const_pool.tile([P, 4, P], bf16)
            nc.gpsimd.dma_start(band_t[:], bands_d.ap())

            # zero the pad columns of every padded operand buffer
            # (vector-engine memsets: a pile of tiny SWDGE DMAs would
            # serialize ~40us ahead of the first input loads)
            for buf in ul_bufs + ucast_bufs + fx_bufs + fy_bufs:
                nc.vector.memset(buf[:, 0:1], 0.0)
                nc.vector.memset(buf[:, WPAD - 1:WPAD], 0.0)

            SY = band_t[:, 0, :]
            SYN = band_t[:, 1, :]
            DY = band_t[:, 2, :]
            DY2 = band_t[:, 3, :]

            def conv_mms(psum, terms):
                """psum[:, :] = sum_i band_i @ src_i(x + dx_i), in 512-halves."""
                for h in (0, 512):
                    n = len(terms)
                    for i, (b, src, dx) in enumerate(terms):
                        nc.tensor.matmul(
                            psum[:, h:h + 512], b,
                            src[:, h + dx + 1:h + dx + 513],
                            start=(i == 0), stop=(i == n - 1))

            lc = 0   # load ring counter
            cc = 0   # cast ring counter
            fc = 0   # flux buffer rotation counter
            state = {}   # stripe index -> (bf16 conv input, fp32 u tile)
            pend = {}    # (stripe, iter) -> tiles dict between phases

            def load_stripe(s):
                nonlocal lc
                img, in_lo, _, _ = stripes[s]
                u_b = ul_bufs[lc % N_ULOAD]; lc += 1
                nc.gpsimd.dma_start(u_b[:, 1:W + 1],
                                    img_d.ap()[img, in_lo:in_lo + P, :])
                u_f = ew_pool.tile([P, W], f32, tag="uin", bufs=6,
                                   name="uin")
                nc.sync.dma_start(u_f[:], img_d.ap()[img, in_lo:in_lo + P, :])
                state[s] = (u_b, u_f)

            def do_a1(s, it):
                """gradient matmuls + squares + s-add."""
                par = s % 2
                u_b, u_f = state[s]

                gx = ps_pool.tile([P, W], f32, tag=f"g{par}", name=f"gx{par}")
                gy = ps_pool.tile([P, W], f32, tag=f"y{par}", name=f"gy{par}")
                conv_mms(gx, [(SY, u_b, 1), (SYN, u_b, -1)])
                conv_mms(gy, [(DY2, u_b, 0), (DY, u_b, -1), (DY, u_b, 1)])

                q1 = ew_pool.tile([P, W], f32, tag="q1", bufs=4)
                q2 = ew_pool.tile([P, W], f32, tag="q2", bufs=4)
                s_t = ew_pool.tile([P, W], f32, tag="s", bufs=4)
                # the s-add sits inside the gx/gy PSUM-residency window, so
                # keep it on the faster DVE
                add_eng = nc.vector
                if USE_ACT_RECIP:
                    nc.scalar.activation(q1[:], gx[:], AF.Square)
                    nc.scalar.activation(q2[:], gy[:], AF.Square)
                    add_eng.tensor_tensor(s_t[:], q1[:], q2[:], OP.add)
                else:
                    nc.scalar.activation(q1[:], gx[:], AF.Square, scale=sa)
                    nc.scalar.activation(q2[:], gy[:], AF.Square, scale=sa)
                    add_eng.scalar_tensor_tensor(
                        s_t[:], q1[:], xb, q2[:], op0=OP.add, op1=OP.add)
                pend[(s, it)] = dict(gx=gx, gy=gy, s_t=s_t)

            def do_a2(s, it):
                """reciprocal + fluxes (consumes gx/gy from psum)."""
                nonlocal fc
                p = pend[(s, it)]
                c_t = ew_pool.tile([P, W], f32, tag="c", bufs=4)
                if USE_ACT_RECIP:
                    _act_recip(nc, c_t[:], p["s_t"][:], scale=rs, bias=rb)
                    flux_s = 1.0
                else:
                    nc.vector.reciprocal_approx_fast(c_t[:], p["s_t"][:])
                    flux_s = dt64
                fx = fx_bufs[fc % N_FBUF]
                fy = fy_bufs[fc % N_FBUF]; fc += 1
                nc.vector.scalar_tensor_tensor(
                    fx[:, 1:W + 1], c_t[:], flux_s, p["gx"][:],
                    op0=OP.mult, op1=OP.mult)
                nc.vector.scalar_tensor_tensor(
                    fy[:, 1:W + 1], c_t[:], flux_s, p["gy"][:],
                    op0=OP.mult, op1=OP.mult)
                p["fx"] = fx; p["fy"] = fy

            def do_b_div(s, it):
                """divergence matmuls (consume the fluxes)."""
                par = s % 2
                p = pend[(s, it)]
                # dv reuses gx's psum slot (same tag, freed by the fx read)
                dv = ps_pool.tile([P, W], f32, tag=f"g{par}", name=f"dv{par}")
                conv_mms(dv, [(SY, p["fx"], 1), (SYN, p["fx"], -1),
                              (DY2, p["fy"], 0), (DY, p["fy"], -1),
                              (DY, p["fy"], 1)])
                p["dv"] = dv

            def do_b_fin(s, it):
                """u update + bf16 requant (or store)."""
                nonlocal cc
                img, in_lo, op_lo, n_out = stripes[s]
                _, u_f = state[s]
                p = pend.pop((s, it))
                dv = p["dv"]

                if it < NUM_ITERATIONS - 1:
                    u_n = ew_pool.tile([P, W], f32, tag="unew", bufs=6,
                                       name="unew")
                    nc.vector.scalar_tensor_tensor(
                        u_n[:], dv[:], 1.0, u_f[:], op0=OP.mult, op1=OP.add)
                    u_nb = ucast_bufs[cc % N_UCAST]; cc += 1
                    nc.scalar.copy(u_nb[:, 1:W + 1], u_n[:])
                    state[s] = (u_nb, u_n)
                else:
                    # final iteration: fp32 result into a pool tile (the
                    # ring tracks the store DMA as a reader, so the slot
                    # is not recycled before the transfer completed)
                    u_o = ew_pool.tile([P, W], f32, tag="uout", bufs=6,
                                       name="uout")
                    nc.vector.scalar_tensor_tensor(
                        u_o[:], dv[:], 1.0, u_f[:], op0=OP.mult, op1=OP.add)
                    r0 = in_lo + op_lo
                    nc.sync.dma_start(
                        out_d.ap()[img, r0:r0 + n_out, :],
                        u_o[op_lo:op_lo + n_out, :])

            # rolling pipeline: per step, divergence phases of last step's
            # chains first (their fluxes are ready), then gradient phases
            # of this step's chains, then reciprocal+flux phases -- keeps
            # every engine queue free of head-of-line blocking.
            def valid(s, it):
                return 0 <= s < n_stripes
            for s in range(min(LOOKAHEAD, n_stripes)):
                load_stripe(s)
            for k in range(n_stripes + 3):
                b_list = [(s, it) for (s, it) in
                          [(k - 3, 2), (k - 2, 1), (k - 1, 0)]
                          if valid(s, it) and (s, it) in pend]
                for (s, it) in b_list:
                    do_b_div(s, it)
                a_list = [(s, it) for (s, it) in
                          [(k - 2, 2), (k - 1, 1), (k, 0)] if valid(s, it)]
                a_list = [(s, it) for (s, it) in a_list
                          if it == 0 or (s, it - 1) not in pend]
                # interleave the u'-finishes with the gradient phases so
                # each engine's in-order queue matches data-readiness
                for i in range(max(len(b_list), len(a_list))):
                    if i < len(b_list):
                        do_b_fin(*b_list[i])
                    if i < len(a_list):
                        do_a1(*a_list[i])
                if k + LOOKAHEAD < n_stripes:
                    load_stripe(k + LOOKAHEAD)
                for (s, it) in a_list:
                    do_a2(s, it)

    nc.compile()
    return nc


def _act_recip(nc, out_ap, in_ap, scale, bias):
    """c = 1/(scale*in + bias) on the scalar engine.

    bass's activation() wrapper refuses Reciprocal outright (a generic
    accuracy warning); the LUT's ~1e-5 relative error (measured on hw
    for this input range) is far inside this problem's tolerance, so
    emit the InstActivation directly.
    """
    from concourse import mybir

    eng = nc.scalar
    imm = lambda v: mybir.ImmediateValue(dtype=mybir.dt.float32, value=float(v))
    return eng.add_instruction(
        mybir.InstActivation(
            name=eng.bass.get_next_instruction_name(),
            func=mybir.ActivationFunctionType.Reciprocal,
            ins=[eng.lower_ap(in_ap), imm(bias), imm(scale), imm(0.0)],
            outs=[eng.lower_ap(out_ap)],
        )
    )


def _get_program():
    if "nc" not in _CACHE:
        _CACHE["nc"] = _build_program()
        _CACHE["bands"] = _band_matrices()
        _CACHE["zeros"] = np.zeros((P, 1), np.float32)
    return _CACHE["nc"], _CACHE["bands"], _CACHE["zeros"]


def kernel(image):
    from concourse.bass_utils import run_bass_kernel_spmd

    image = np.asarray(image)
    orig_shape = image.shape          # (16, 1, 1024, 1024)
    flat = np.ascontiguousarray(
        image.reshape(N_CORES, IMGS_PER_CORE, H, W).astype(np.float32))

    nc, bands, zeros = _get_program()
    in_maps = [{"image": flat[c], "bands": bands, "zeros": zeros}
               for c in range(N_CORES)]
    res = run_bass_kernel_spmd(nc, in_maps, core_ids=list(range(N_CORES)))
    out = np.stack([res.results[c]["out"] for c in range(N_CORES)])
    return out.reshape(orig_shape).astype(np.float32)


# revision 16
# speedup vs baseline: 1.9146x; 1.0043x over previous
"""Perona-Malik nonlinear diffusion (3 iterations) on Trainium2, 8-core SPMD.

Input : image (16, 1, 1024, 1024) float32
Output: same shape, after 3 iterations of
    g  = Sobel gradients (SAME/zero padding)
    c  = 1 / (1 + |g|^2/kappa^2)   (sqrt folded out algebraically)
    u += DT * div(c * g)           (div via the same Sobel stencils)

Sharding: pure data-parallel, 2 images per NeuronCore.

Per-core kernel: images processed in 9 stripes of 128 rows; all 3
iterations run in SBUF (halo recompute; image top/bottom edges aligned
to tile edges so band-matrix truncation implements the zero padding in
y).  The y-direction Sobel parts are band-matrix matmuls on the tensor
engine accumulating in PSUM; x-direction parts are free-dim shifted
views of zero-padded operand tiles.  All matmul operands are bf16
(full-rate moving operand - fp32r runs at half rate on trn2 - and
fast weight loads); the bands are exact small integers in bf16, and
the running image u is kept in fp32 on the side so bf16 rounding does
not accumulate across iterations (only the conv inputs are quantized).

Engine split per stripe-iteration (all tiles [128, 1024]):
  PE : 20 matmuls (gx 2+2, gy 3+3 halves, divergence 5+5), bf16
  ACT: q1 = Square(gx), q2 = Square(gy), c = Reciprocal(rs*s + rb)
       (square and reciprocal share one activation-table set -> the
        table is loaded once, not swapped per call like ln/exp)
  GPS: s = q1 + q2, and the fp32 -> bf16 re-quantization of u'
  DVE: fx = c*gx, fy = c*gy (scalar_tensor_tensor, bf16 out),
       u' = dv + u (fp32)
All DT/kappa scaling is folded into the Reciprocal's affine pre-scale.

Pipelining: stripes are processed in a rolling 3-deep software pipeline
(emit iter2 of stripe s-2, iter1 of s-1, iter0 of s per step) with PSUM
tags assigned by stripe parity: {gx,dv} and {gy} pairs for even/odd
stripes = exactly 8 banks, so the tensor engine always has an
independent stripe's matmuls to run while another stripe's elementwise
chain completes (keeps the PE HAM clock-gate warm).  Input stripe loads
(SWDGE cast f32->bf16 on gpsimd + plain f32 on sync) are prefetched a
few stripes ahead; output stores ride the sync queue and read from pool
tiles so the ring slot is not recycled before the DMA drains.
"""

import numpy as np

NUM_ITERATIONS = 3
KAPPA = 0.05
DT = 0.25
EPS = 1e-8

N_CORES = 8
IMGS_PER_CORE = 2
H = 1024
W = 1024
P = 128          # partitions / stripe rows
HALO = 6         # 2 rows shrink per iteration * 3 iterations
VALID = P - 2 * HALO          # 116 valid output rows per mid stripe
WPAD = W + 2                  # one zero pad column each side

N_ULOAD = 6      # bf16 input-load ring (loads prefetched LOOKAHEAD ahead)
N_UCAST = 6      # bf16 u' re-quantization ring
N_FBUF = 8
LOOKAHEAD = 3    # stripes of input-DMA prefetch

# knobs (fallbacks if an experiment fails on hw)
USE_ACT_RECIP = True    # False -> nc.vector.reciprocal_approx_fast on DVE
S_ADD_ON_GPS = True     # False -> s-add on DVE
CAST_ON_GPS = False     # False -> u' bf16 cast on ACT

_CACHE = {}


def _band_matrices():
    """[128, 4, 128] float32, index [k, band, m] = lhsT[k, m] of band matrix.

    out[m, :] = sum_k lhsT[k, m] * u[k, :]
      band 0 (Sy):  u[m-1] + 2u[m] + u[m+1]
      band 1 (Syn): -Sy
      band 2 (Dy):  u[m+1] - u[m-1]
      band 3 (Dy2): 2*Dy
    """
    sy = np.zeros((P, P), np.float32)
    dy = np.zeros((P, P), np.float32)
    for m in range(P):
        sy[m, m] = 2.0
        if m - 1 >= 0:
            sy[m - 1, m] = 1.0   # k = m-1
            dy[m - 1, m] = -1.0
        if m + 1 < P:
            sy[m + 1, m] = 1.0   # k = m+1
            dy[m + 1, m] = 1.0
    bands = np.stack([sy, -sy, dy, 2.0 * dy], axis=1)  # [k, 4, m]
    return np.ascontiguousarray(bands.astype(np.float32))


def _stripes():
    """(img, input_row_lo, out_partition_lo, n_out) per stripe."""
    per_img = [(0, 0, P - HALO)]
    pos = P - HALO
    last_start = H - P
    while pos < last_start + HALO:
        n = min(VALID, last_start + HALO - pos)
        per_img.append((pos - HALO, HALO, n))
        pos += n
    per_img.append((last_start, pos - last_start, H - pos))
    return [(img,) + s for img in range(IMGS_PER_CORE) for s in per_img]


def _build_program():
    import concourse.bacc as bacc
    import concourse.tile as tile
    from concourse import mybir

    f32 = mybir.dt.float32
    bf16 = mybir.dt.bfloat16
    AF = mybir.ActivationFunctionType
    OP = mybir.AluOpType

    # c'' = (DT/64) / (1 + (s/64 + eps)/kappa^2)  with s = gx^2+gy^2 in
    # integer-Sobel units (x8 per conv).  1/c'' = s*rs + rb:
    rs = 1.0 / (DT * KAPPA * KAPPA)
    rb = (64.0 / DT) * (1.0 + EPS / (KAPPA * KAPPA))
    # plan-A constants (reciprocal on DVE): x = q1 + q2 + xb with
    # q = (sa*g)^2, then c = 1/x and fx = (c*dt64)*gx.
    sa = 1.0 / (8.0 * KAPPA)
    xb = 1.0 + EPS / (KAPPA * KAPPA)
    dt64 = DT / 64.0

    nc = bacc.Bacc("TRN2", target_bir_lowering=False, debug=False)

    img_d = nc.dram_tensor("image", [IMGS_PER_CORE, H, W], f32, kind="ExternalInput")
    bands_d = nc.dram_tensor("bands", [P, 4, P], f32, kind="ExternalInput")
    zeros_d = nc.dram_tensor("zeros", [P, 1], f32, kind="ExternalInput")
    out_d = nc.dram_tensor("out", [IMGS_PER_CORE, H, W], f32, kind="ExternalOutput")

    # static padded bf16 conv operands (pads zeroed once, never rewritten).
    # Loads and u'-casts use separate rings: a load for stripe s+LOOKAHEAD
    # is emitted before stripe s's compute, so a shared ring would recycle
    # a slot whose reader is not yet emitted (the dependency tracker can
    # only order against already-emitted instructions).
    ul_bufs = [nc.alloc_sbuf_tensor(f"ul{i}", [P, WPAD], bf16).ap()
               for i in range(N_ULOAD)]
    ucast_bufs = [nc.alloc_sbuf_tensor(f"uc{i}", [P, WPAD], bf16).ap()
                  for i in range(N_UCAST)]
    fx_bufs = [nc.alloc_sbuf_tensor(f"fxb{i}", [P, WPAD], bf16).ap()
               for i in range(N_FBUF)]
    fy_bufs = [nc.alloc_sbuf_tensor(f"fyb{i}", [P, WPAD], bf16).ap()
               for i in range(N_FBUF)]

    stripes = _stripes()
    n_stripes = len(stripes)

    with tile.TileContext(nc) as tc:
        with (
            tc.tile_pool(name="const", bufs=1) as const_pool,
            tc.tile_pool(name="ew", bufs=3) as ew_pool,
            tc.tile_pool(name="ps", bufs=1, space="PSUM") as ps_pool,
        ):
            band_t = const_pool.tile([P, 4, P], bf16)
            nc.gpsimd.dma_start(band_t[:], bands_d.ap())

            # zero the pad columns of every padded operand buffer
            # (vector-engine memsets: a pile of tiny SWDGE DMAs would
            # serialize ~40us ahead of the first input loads)
            for buf in ul_bufs + ucast_bufs + fx_bufs + fy_bufs:
                nc.vector.memset(buf[:, 0:1], 0.0)
                nc.vector.memset(buf[:, WPAD - 1:WPAD], 0.0)

            SY = band_t[:, 0, :]
            SYN = band_t[:, 1, :]
            DY = band_t[:, 2, :]
            DY2 = band_t[:, 3, :]

            def conv_mms(psum, terms):
                """psum[:, :] = sum_i band_i @ src_i(x + dx_i), in 512-halves."""
                for h in (0, 512):
                    n = len(terms)
                    for i, (b, src, dx) in enumerate(terms):
                        nc.tensor.matmul(
                            psum[:, h:h + 512], b,
                            src[:, h + dx + 1:h + dx + 513],
                            start=(i == 0), stop=(i == n - 1))

            lc = 0   # load ring counter
            cc = 0   # cast ring counter
            fc = 0   # flux buffer rotation counter
            state = {}   # stripe index -> (bf16 conv input, fp32 u tile)
            pend = {}    # (stripe, iter) -> tiles dict between phases

            def load_stripe(s):
                nonlocal lc
                img, in_lo, _, _ = stripes[s]
                u_b = ul_bufs[lc % N_ULOAD]; lc += 1
                nc.gpsimd.dma_start(u_b[:, 1:W + 1],
                                    img_d.ap()[img, in_lo:in_lo + P, :])
                u_f = ew_pool.tile([P, W], f32, tag="uin", bufs=6,
                                   name="uin")
                nc.sync.dma_start(u_f[:], img_d.ap()[img, in_lo:in_lo + P, :])
                state[s] = (u_b, u_f)

            def do_a1(s, it):
                """gradient matmuls + squares + s-add."""
                par = s % 2
                u_b, u_f = state[s]

                gx = ps_pool.tile([P, W], f32, tag=f"g{par}", name=f"gx{par}")
                gy = ps_pool.tile([P, W], f32, tag=f"y{par}", name=f"gy{par}")
                conv_mms(gx, [(SY, u_b, 1), (SYN, u_b, -1)])
                conv_mms(gy, [(DY2, u_b, 0), (DY, u_b, -1), (DY, u_b, 1)])

                q1 = ew_pool.tile([P, W], f32, tag="q1", bufs=4)
                q2 = ew_pool.tile([P, W], f32, tag="q2", bufs=4)
                s_t = ew_pool.tile([P, W], f32, tag="s", bufs=4)
                # the s-add sits inside the gx/gy PSUM-residency window, so
                # keep it on the faster DVE
                add_eng = nc.vector
                if USE_ACT_RECIP:
                    nc.scalar.activation(q1[:], gx[:], AF.Square)
                    nc.scalar.activation(q2[:], gy[:], AF.Square)
                    add_eng.tensor_tensor(s_t[:], q1[:], q2[:], OP.add)
                else:
                    nc.scalar.activation(q1[:], gx[:], AF.Square, scale=sa)
                    nc.scalar.activation(q2[:], gy[:], AF.Square, scale=sa)
                    add_eng.scalar_tensor_tensor(
                        s_t[:], q1[:], xb, q2[:], op0=OP.add, op1=OP.add)
                pend[(s, it)] = dict(gx=gx, gy=gy, s_t=s_t)

            def do_a2(s, it):
                """reciprocal + fluxes (consumes gx/gy from psum)."""
                nonlocal fc
                p = pend[(s, it)]
                c_t = ew_pool.tile([P, W], f32, tag="c", bufs=4)
                if USE_ACT_RECIP:
                    _act_recip(nc, c_t[:], p["s_t"][:], scale=rs, bias=rb)
                    flux_s = 1.0
                else:
                    nc.vector.reciprocal_approx_fast(c_t[:], p["s_t"][:])
                    flux_s = dt64
                fx = fx_bufs[fc % N_FBUF]
                fy = fy_bufs[fc % N_FBUF]; fc += 1
                nc.vector.scalar_tensor_tensor(
                    fx[:, 1:W + 1], c_t[:], flux_s, p["gx"][:],
                    op0=OP.mult, op1=OP.mult)
                nc.vector.scalar_tensor_tensor(
                    fy[:, 1:W + 1], c_t[:], flux_s, p["gy"][:],
                    op0=OP.mult, op1=OP.mult)
                p["fx"] = fx; p["fy"] = fy

            def do_b_div(s, it):
                """divergence matmuls (consume the fluxes)."""
                par = s % 2
                p = pend[(s, it)]
                # dv reuses gx's psum slot (same tag, freed by the fx read)
                dv = ps_pool.tile([P, W], f32, tag=f"g{par}", name=f"dv{par}")
                conv_mms(dv, [(SY, p["fx"], 1), (SYN, p["fx"], -1),
                              (DY2, p["fy"], 0), (DY, p["fy"], -1),
                              (DY, p["fy"], 1)])
                p["dv"] = dv

            def do_b_fin(s, it):
                """u update + bf16 requant (or store)."""
                nonlocal cc
                img, in_lo, op_lo, n_out = stripes[s]
                _, u_f = state[s]
                p = pend.pop((s, it))
                dv = p["dv"]

                if it < NUM_ITERATIONS - 1:
                    u_n = ew_pool.tile([P, W], f32, tag="unew", bufs=6,
                                       name="unew")
                    nc.vector.scalar_tensor_tensor(
                        u_n[:], dv[:], 1.0, u_f[:], op0=OP.mult, op1=OP.add)
                    u_nb = ucast_bufs[cc % N_UCAST]; cc += 1
                    nc.scalar.copy(u_nb[:, 1:W + 1], u_n[:])
                    state[s] = (u_nb, u_n)
                else:
                    # final iteration: fp32 result into a pool tile (the
                    # ring tracks the store DMA as a reader, so the slot
                    # is not recycled before the transfer completed)
                    u_o = ew_pool.tile([P, W], f32, tag="uout", bufs=6,
                                       name="uout")
                    nc.vector.scalar_tensor_tensor(
                        u_o[:], dv[:], 1.0, u_f[:], op0=OP.mult, op1=OP.add)
                    r0 = in_lo + op_lo
                    nc.sync.dma_start(
                        out_d.ap()[img, r0:r0 + n_out, :],
                        u_o[op_lo:op_lo + n_out, :])

            # rolling pipeline: per step, divergence phases of last step's
            # chains first (their fluxes are ready), then gradient phases
            # of this step's chains, then reciprocal+flux phases -- keeps
            # every engine queue free of head-of-line blocking.
            def valid(s, it):
                return 0 <= s < n_stripes
            for s in range(min(LOOKAHEAD, n_stripes)):
                load_stripe(s)
            for k in range(n_stripes + 3):
                b_list = [(s, it) for (s, it) in
                          [(k - 3, 2), (k - 2, 1), (k - 1, 0)]
                          if valid(s, it) and (s, it) in pend]
                for (s, it) in b_list:
                    do_b_div(s, it)
                a_list = [(s, it) for (s, it) in
                          [(k - 2, 2), (k - 1, 1), (k, 0)] if valid(s, it)]
                a_list = [(s, it) for (s, it) in a_list
                          if it == 0 or (s, it - 1) not in pend]
                # interleave the u'-finishes with the gradient phases so
                # each engine's in-order queue matches data-readiness;
                # a1 of (s, it) must follow b_fin of (s, it-1), which sits
                # one position later in b_list
                if b_list:
                    do_b_fin(*b_list[0])
                for i in range(max(len(b_list) - 1, len(a_list))):
                    if i + 1 < len(b_list):
                        do_b_fin(*b_list[i + 1])
                    if i < len(a_list):
                        do_a1(*a_list[i])
                if k + LOOKAHEAD < n_stripes:
                    load_stripe(k + LOOKAHEAD)
                for (s, it) in a_list:
                    do_a2(s, it)

    nc.compile()
    return nc


def _act_recip(nc, out_ap, in_ap, scale, bias):
    """c = 1/(scale*in + bias) on the scalar engine.

    bass's activation() wrapper refuses Reciprocal outright (a generic
    accuracy warning); the LUT's ~1e-5 relative error (measured on hw
    for this input range) is far inside this problem's tolerance, so
    emit the InstActivation directly.
    """
    from concourse import mybir

    eng = nc.scalar
    imm = lambda v: mybir.ImmediateValue(dtype=mybir.dt.float32, value=float(v))
    return eng.add_instruction(
        mybir.InstActivation(
            name=eng.bass.get_next_instruction_name(),
            func=mybir.ActivationFunctionType.Reciprocal,
            ins=[eng.lower_ap(in_ap), imm(bias), imm(scale), imm(0.0)],
            outs=[eng.lower_ap(out_ap)],
        )
    )


def _get_program():
    if "nc" not in _CACHE:
        _CACHE["nc"] = _build_program()
        _CACHE["bands"] = _band_matrices()
        _CACHE["zeros"] = np.zeros((P, 1), np.float32)
    return _CACHE["nc"], _CACHE["bands"], _CACHE["zeros"]


def kernel(image):
    from concourse.bass_utils import run_bass_kernel_spmd

    image = np.asarray(image)
    orig_shape = image.shape          # (16, 1, 1024, 1024)
    flat = np.ascontiguousarray(
        image.reshape(N_CORES, IMGS_PER_CORE, H, W).astype(np.float32))

    nc, bands, zeros = _get_program()
    in_maps = [{"image": flat[c], "bands": bands, "zeros": zeros}
               for c in range(N_CORES)]
    res = run_bass_kernel_spmd(nc, in_maps, core_ids=list(range(N_CORES)))
    out = np.stack([res.results[c]["out"] for c in range(N_CORES)])
    return out.reshape(orig_shape).astype(np.float32)
